# revision 2
# baseline (speedup 1.0000x reference)
"""MoE top-1 routing kernel for Trainium2 (8 NeuronCores, expert-parallel).

Strategy:
  - Gate (x @ Wg.T + bg, argmax) is computed on host in float64. The min
    top-2 logit gap for this problem's data is ~1.2e-5, orders of magnitude
    above any fp32 backend's rounding noise (~1e-6), so the fp64 argmax
    matches the fp32 reference argmax exactly.
  - Tokens are grouped by expert on host (the "all-to-all dispatch");
    core e receives expert e's tokens (capacity-padded) plus expert e's
    weights, and runs the dense SwiGLU FFN for just those tokens.
  - Outputs are scattered back to token order on host (the "combine").
    With top-1 routing the combine weight is exactly 1.0.

Device kernel (per core), all matmuls on the PE array:
  h1^T = W1 x^T   (contract D, f on partitions)
  h2^T = W2 x^T
  g^T  = silu(h1^T) * h2^T
  y^T  = W3 g^T    (contract F, d on partitions)
All tensors are staged transposed (feature-major) so the PE contraction
dim always sits on partitions; the host does the transposes.
"""

import sys
from contextlib import ExitStack

if "/opt/trn_rl_repo" not in sys.path:
    sys.path.insert(0, "/opt/trn_rl_repo")

import numpy as np

P = 128
D = 768          # model dim
E = 8            # experts == cores
F = 469          # ffn hidden
FP = 512         # F padded to a multiple of 128
KT = D // P      # 6 k-tiles over D
MT = FP // P     # 4 f-tiles over padded F
DT = D // P      # 6 out-tiles over D
MIN_C = 128                # capacity floor; actual C adapts to max expert load
CHUNK = 512                # moving-operand free dim per matmul

# "float32" | "float32r" | "bfloat16" — matmul input precision on device.
MM_MODE = "bfloat16"

# pool buffer counts (tunable)
BUFS = {"x": 3, "g": 2, "s": 4, "o": 6, "ps": 8}
CHUNK_SIZES = None   # explicit chunk-size list override (else balanced split)
A_GROUP = 2          # f-tiles accumulated concurrently in stage A (1, 2, or 4)
B_SPLIT = False      # start stage-B early during pair 1 (hurts: PSUM pressure)
WARMUP_MMS = 8      # dummy matmuls during the DMA preload to pre-warm the PE clock
W3_HALVES = True     # load w3 in two d-halves so stage B starts sooner
# DMA plumbing knobs
X_MERGE = False       # one merged x DMA per chunk (vs 6 per-k DMAs)
W_MERGE = False       # single DMA each for W2/W3 (vs per-k/-m)
STORE_GPSIMD = False  # stores via SWDGE/Pool (vs HWDGE/sync)

_cache = {}


def _np_in_dtype():
    if MM_MODE == "bfloat16":
        import ml_dtypes

        return np.dtype(ml_dtypes.bfloat16)
    return np.dtype(np.float32)


def _build(C):
    """Build + compile the per-core Tile kernel for capacity C tokens."""
    import concourse.bacc as bacc
    import concourse.tile as tile
    from concourse import mybir

    f32 = mybir.dt.float32
    # float32r = fp32 bytes, reduced-precision PE multiply (full matmul rate
    # at >=256 moving columns vs fp32's 1/4 rate; rel-err ~2e-4 on this net).
    # Declared natively so the BIR verifier sees f32r producers end-to-end.
    in_dt = {
        "bfloat16": mybir.dt.bfloat16,
        "float32r": mybir.dt.float32r,
        "float32": mybir.dt.float32,
    }[MM_MODE]

    def mm_view(ap):
        return ap

    nc = bacc.Bacc("TRN2", target_bir_lowering=False, debug=False, num_devices=E)

    xt = nc.dram_tensor("xt", [KT, P, C], in_dt, kind="ExternalInput").ap()
    w1t = nc.dram_tensor("w1t", [KT, P, FP], in_dt, kind="ExternalInput").ap()
    w2t = nc.dram_tensor("w2t", [KT, P, FP], in_dt, kind="ExternalInput").ap()
    w3t = nc.dram_tensor("w3t", [MT, P, D], in_dt, kind="ExternalInput").ap()
    yt = nc.dram_tensor("yt", [DT, P, C], f32, kind="ExternalOutput").ap()

    # Balanced chunk split: f32r matmuls drop to 1/4 rate below 256 columns,
    # so keep every chunk >= 256 (when C allows) instead of a ragged 512-tail.
    # Chunk sizes must be EVEN: odd moving-column counts fail the walrus
    # is_valid_s3d3_mm ISA check for 4-byte matmul dtypes.
    assert C % 2 == 0, C
    if CHUNK_SIZES is not None:
        sizes = list(CHUNK_SIZES)
    else:
        nch = max(1, -(-C // CHUNK))
        u, uextra = divmod(C // 2, nch)
        sizes = [2 * (u + (1 if i < uextra else 0)) for i in range(nch)]
    chunks = []
    off = 0
    for nn in sizes:
        chunks.append((off, nn))
        off += nn
    assert off == C, (off, C)

    silu = mybir.ActivationFunctionType.Silu

    with tile.TileContext(nc) as tc, ExitStack() as ctx:
        wpool = ctx.enter_context(tc.tile_pool(name="w", bufs=1))
        xpool = ctx.enter_context(tc.tile_pool(name="x", bufs=BUFS["x"]))
        gpool = ctx.enter_context(tc.tile_pool(name="g", bufs=BUFS["g"]))
        spool = ctx.enter_context(tc.tile_pool(name="s", bufs=BUFS["s"]))
        opool = ctx.enter_context(tc.tile_pool(name="o", bufs=BUFS["o"]))
        # one shared PSUM tag: 8 slots = all 8 banks; stage A holds up to 8
        # accumulators (h1 x4, h2 x4), stage B grabs slots as they free
        pspool = ctx.enter_context(
            tc.tile_pool(name="ps", bufs=BUFS["ps"], space="PSUM")
        )

        w1_sb = [
            wpool.tile([P, FP], in_dt, tag=f"w1_{k}", name=f"w1_{k}")
            for k in range(KT)
        ]

        def w1s(k, m):
            return w1_sb[k][:, m * P : (m + 1) * P]

        if W_MERGE:
            w2_sb = wpool.tile([P, KT, FP], in_dt, tag="w2", name="w2_sb")
            w3_sb = wpool.tile([P, MT, D], in_dt, tag="w3", name="w3_sb")
            w2s = lambda k: w2_sb[:, k, :]
            w3s = lambda m: w3_sb[:, m, :]
        else:
            w2_l = [
                wpool.tile([P, FP], in_dt, tag=f"w2_{k}", name=f"w2_{k}")
                for k in range(KT)
            ]
            w3_l = [
                wpool.tile([P, D], in_dt, tag=f"w3_{m}", name=f"w3_{m}")
                for m in range(MT)
            ]
            w2s = lambda k: w2_l[k][:]
            w3s = lambda m: w3_l[m][:]

        # dummy matmuls on a zeroed tile fill the DMA-preload window so the
        # PE clock ramp (cold 1.2GHz -> warm 2.4GHz after ~3us sustained) is
        # already paid before the first real matmul; outputs are never used
        if WARMUP_MMS:
            warm = wpool.tile([P, 256], in_dt, tag="warm", name="warm")
            nc.vector.memset(warm[:], 0.0)
            wps = pspool.tile([P, 256], f32, tag="ps", name="wps")
            for _ in range(WARMUP_MMS):
                nc.tensor.matmul(wps[:], warm[:, :P], warm[:], start=True, stop=True)
            wsink = wpool.tile([P, 256], f32, tag="wsink", name="wsink")
            nc.scalar.copy(wsink[:], wps[:])  # consume so the PSUM slot frees

        # chunk-0 x interleaved with W1 (both split per k) so the k-outer pass
        # starts after the first ~0.5MB of DMA instead of the full preload
        nn0 = chunks[0][1]
        x0 = [
            xpool.tile([P, nn0], in_dt, tag=f"x0_{k}", name=f"x0_{k}")
            for k in range(KT)
        ]
        for k in range(KT):
            nc.sync.dma_start(w1_sb[k][:], w1t[k])
            nc.sync.dma_start(x0[k][:], xt[k, :, 0:nn0])
        if W_MERGE:
            nc.sync.dma_start(w2_sb[:], w2t.rearrange("k p f -> p k f"))
            nc.sync.dma_start(w3_sb[:], w3t.rearrange("m p d -> p m d"))
        else:
            for k in range(KT):
                nc.sync.dma_start(w2s(k), w2t[k])
            for m in range(MT):
                if W3_HALVES:
                    nc.sync.dma_start(w3s(m)[:, : D // 2], w3t[m, :, : D // 2])
                    nc.sync.dma_start(w3s(m)[:, D // 2 :], w3t[m, :, D // 2 :])
                else:
                    nc.sync.dma_start(w3s(m), w3t[m])

        for ci, (n0, nn) in enumerate(chunks):
            if ci == 0:
                xn = x0
            elif X_MERGE:
                xnt = xpool.tile([P, KT, nn], in_dt, tag="xn", name="xnt")
                nc.sync.dma_start(
                    xnt[:], xt[:, :, n0 : n0 + nn].rearrange("k p n -> p k n")
                )
                xn = [xnt[:, k, :] for k in range(KT)]
            else:
                xn = [
                    xpool.tile([P, nn], in_dt, tag=f"xn_{k}", name=f"xn_{k}")
                    for k in range(KT)
                ]
                for k in range(KT):
                    nc.sync.dma_start(xn[k][:], xt[k, :, n0 : n0 + nn])

            # stage A in m-pairs, k-outer inside: only 4 PSUM banks held at
            # a time (vs 8), leaving headroom for stage-B/next-chunk overlap;
            # the k-outer inner order still lets chunk-0 start after the
            # first w1/x k-tile lands
            # number of d-tiles whose stage-B (m=0,1) matmuls are emitted
            # early, between stage-A pairs; capped at 2 so PSUM stays at
            # 2 (open pso) + 4 (pair-1 accumulators) + slack <= 8 banks
            early_d = 2 if (B_SPLIT and A_GROUP == 2 and MT == 4) else 0
            psos = {}
            gs = []
            for mp in range(MT // A_GROUP):
                ms = tuple(range(A_GROUP * mp, A_GROUP * (mp + 1)))
                ps1 = {
                    m: pspool.tile([P, nn], f32, tag="ps", name=f"ps1_{m}")
                    for m in ms
                }
                for k in range(KT):
                    for m in ms:
                        nc.tensor.matmul(
                            ps1[m][:],
                            mm_view(w1s(k, m)),
                            mm_view(xn[k][:]),
                            start=(k == 0),
                            stop=(k == KT - 1),
                        )
                ps2 = {
                    m: pspool.tile([P, nn], f32, tag="ps", name=f"ps2_{m}")
                    for m in ms
                }
                for k in range(KT):
                    for m in ms:
                        nc.tensor.matmul(
                            ps2[m][:],
                            mm_view(w2s(k)[:, m * P : (m + 1) * P]),
                            mm_view(xn[k][:]),
                            start=(k == 0),
                            stop=(k == KT - 1),
                        )
                for m in ms:
                    sil = spool.tile([P, nn], f32, tag="sil", name="sil")
                    nc.scalar.activation(sil[:], ps1[m][:], silu)
                    g = gpool.tile([P, nn], in_dt, tag=f"g{m}", name=f"g{m}")
                    nc.vector.tensor_mul(g[:], sil[:], ps2[m][:])
                    gs.append(g)
                if mp == 0:
                    # overlap: open the first stage-B accumulators using the
                    # already-finished g0/g1 while pair 1 is still on the PE
                    for d in range(early_d):
                        pso = pspool.tile([P, nn], f32, tag="ps", name="pso")
                        psos[d] = pso
                        for m in ms:
                            nc.tensor.matmul(
                                pso[:],
                                mm_view(w3s(m)[:, d * P : (d + 1) * P]),
                                mm_view(gs[m][:]),
                                start=(m == 0),
                                stop=False,
                            )

            for d in range(DT):
                if d in psos:
                    pso = psos[d]
                    rest = range(A_GROUP, MT)
                else:
                    pso = pspool.tile([P, nn], f32, tag="ps", name="pso")
                    rest = range(MT)
                for m in rest:
                    nc.tensor.matmul(
                        pso[:],
                        mm_view(w3s(m)[:, d * P : (d + 1) * P]),
                        mm_view(gs[m][:]),
                        start=(m == 0),
                        stop=(m == MT - 1),
                    )
                ot = opool.tile([P, nn], f32, tag="ot", name="ot")
                # alternate copy engine so the stage-B epilogue isn't
                # serialized on ACT alone
                if d % 2 == 1:
                    nc.scalar.copy(ot[:], pso[:])
                else:
                    nc.vector.tensor_copy(ot[:], pso[:])
                eng = nc.gpsimd if STORE_GPSIMD else nc.sync
                eng.dma_start(yt[d, :, n0 : n0 + nn], ot[:])

    nc.compile()
    return nc


LAST_RESULTS = None  # BassKernelResults of the most recent run (for test harness)


def kernel(x, Wg, bg, W1, W2, W3):
    global LAST_RESULTS
    from concourse.bass_utils import run_bass_kernel_spmd

    x = np.asarray(x)
    Wg, bg = np.asarray(Wg), np.asarray(bg)
    W1, W2, W3 = np.asarray(W1), np.asarray(W2), np.asarray(W3)
    B, S, d = x.shape
    T = B * S
    assert d == D and Wg.shape == (E, D)

    xf = np.ascontiguousarray(x.reshape(T, D))

    # ---- host gate + top-1 routing (fp64: exact vs any fp32 backend) ----
    gate = xf.astype(np.float64) @ Wg.astype(np.float64).T + bg.astype(np.float64)
    eid = np.argmax(gate, axis=1)
    counts = np.bincount(eid, minlength=E)
    order = np.argsort(eid, kind="stable")
    offs = np.concatenate(([0], np.cumsum(counts)))

    C = max(MIN_C, 2 * int(-(-counts.max() // 2)))
    key = (C, MM_MODE)
    if key not in _cache:
        _cache[key] = _build(C)
    nc = _cache[key]

    in_dt = _np_in_dtype()

    # ---- build per-core inputs (dispatch) ----
    in_maps = []
    tok_lists = []
    for e in range(E):
        toks = order[offs[e] : offs[e + 1]]
        tok_lists.append(toks)
        ce = len(toks)
        xeT = np.zeros((D, C), dtype=in_dt)
        if ce:
            xeT[:, :ce] = xf[toks].T.astype(in_dt)
        w1 = np.zeros((D, FP), dtype=in_dt)
        w1[:, :F] = W1[e].T.astype(in_dt)
        w2 = np.zeros((D, FP), dtype=in_dt)
        w2[:, :F] = W2[e].T.astype(in_dt)
        w3 = np.zeros((FP, D), dtype=in_dt)
        w3[:F, :] = W3[e].T.astype(in_dt)
        in_maps.append(
            {
                "xt": np.ascontiguousarray(xeT.reshape(KT, P, C)),
                "w1t": np.ascontiguousarray(w1.reshape(KT, P, FP)),
                "w2t": np.ascontiguousarray(w2.reshape(KT, P, FP)),
                "w3t": np.ascontiguousarray(w3.reshape(MT, P, D)),
            }
        )

    res = run_bass_kernel_spmd(nc, in_maps, list(range(E)))
    LAST_RESULTS = res

    # ---- combine: scatter outputs back to token order ----
    y = np.empty((T, D), dtype=np.float32)
    for e in range(E):
        toks = tok_lists[e]
        if len(toks):
            yte = res.results[e]["yt"].reshape(D, C)
            y[toks] = yte[:, : len(toks)].T
    return y.reshape(B, S, d)



# revision 5
# speedup vs baseline: 1.0290x; 1.0290x over previous
"""MoE top-1 routing kernel for Trainium2 (8 NeuronCores, expert-parallel).

Strategy:
  - Gate (x @ Wg.T + bg, argmax) is computed on host in float64. The min
    top-2 logit gap for this problem's data is ~1.2e-5, orders of magnitude
    above any fp32 backend's rounding noise (~1e-6), so the fp64 argmax
    matches the fp32 reference argmax exactly.
  - Tokens are grouped by expert on host (the "all-to-all dispatch");
    core e receives expert e's tokens (capacity-padded) plus expert e's
    weights, and runs the dense SwiGLU FFN for just those tokens.
  - Outputs are scattered back to token order on host (the "combine").
    With top-1 routing the combine weight is exactly 1.0.

Device kernel (per core), all matmuls on the PE array in bf16 (full
1 cycle/row rate at any moving width; rel-err ~4e-3 end to end):
  h1^T = W1 x^T   (contract D, f on partitions)
  h2^T = W2 x^T
  g^T  = silu(h1^T) * h2^T
  y^T  = W3 g^T    (contract F, d on partitions)
All tensors are staged transposed (feature-major) so the PE contraction
dim always sits on partitions; the host does the transposes.

Schedule notes (cost-model driven):
  - HWDGE serializes every DMA at ~625ns regardless of size, so loads are
    merged into k-pair / whole-tensor DMAs and stores into one merged DMA
    per chunk.
  - The PE clock ramps (1.54 -> 0.83 -> 0.417 ns/row) and reaches full
    speed only after 3us of continuous busy; dummy warmup matmuls burn the
    ramp during the DMA preload window.
  - The last chunk is small (128 tokens) and its d-tile stores fan out
    across the SP/Act/DVE issue queues so the end-of-kernel store+drain
    tail shrinks.
"""

import sys
from contextlib import ExitStack

if "/opt/trn_rl_repo" not in sys.path:
    sys.path.insert(0, "/opt/trn_rl_repo")

import numpy as np

P = 128
D = 768          # model dim
E = 8            # experts == cores
F = 469          # ffn hidden
FP = 512         # F padded to a multiple of 128
KT = D // P      # 6 k-tiles over D
MT = FP // P     # 4 f-tiles over padded F
DT = D // P      # 6 out-tiles over D
KP = KT // 2     # 3 k-pairs (DMA granularity for weight/x0 preload)
MIN_C = 128      # capacity floor; actual C adapts to max expert load
CHUNK = 512      # moving-operand free dim per matmul (== one PSUM bank f32)
TAIL = 128       # final chunk size (small => short store tail)

MM_MODE = "bfloat16"   # matmul input precision on device
WARMUP_MMS = 14        # dummy matmuls to pre-warm the PE clock during preload
WARM_COLS = 256

# pool buffer counts
BUFS = {"x": 1, "g": 2, "s": 4, "o": 2, "ps": 8}

_cache = {}


def _np_in_dtype():
    if MM_MODE == "bfloat16":
        import ml_dtypes

        return np.dtype(ml_dtypes.bfloat16)
    return np.dtype(np.float32)


def _chunk_sizes(C):
    """[~512]*n + [TAIL]; all even, sum == C."""
    assert C % 2 == 0, C
    if C <= CHUNK:
        return [C]
    body = C - TAIL
    nb = -(-body // CHUNK)
    u, rem = divmod(body // 2, nb)
    sizes = [2 * (u + (1 if i < rem else 0)) for i in range(nb)]
    sizes.append(TAIL)
    assert sum(sizes) == C
    return sizes


def _build(C):
    """Build + compile the per-core Tile kernel for capacity C tokens."""
    import concourse.bacc as bacc
    import concourse.tile as tile
    from concourse import mybir

    f32 = mybir.dt.float32
    in_dt = {
        "bfloat16": mybir.dt.bfloat16,
        "float32r": mybir.dt.float32r,
        "float32": mybir.dt.float32,
    }[MM_MODE]

    nc = bacc.Bacc("TRN2", target_bir_lowering=False, debug=False, num_devices=E)

    xt = nc.dram_tensor("xt", [KT, P, C], in_dt, kind="ExternalInput").ap()
    w1t = nc.dram_tensor("w1t", [KT, P, FP], in_dt, kind="ExternalInput").ap()
    w2t = nc.dram_tensor("w2t", [KT, P, FP], in_dt, kind="ExternalInput").ap()
    w3t = nc.dram_tensor("w3t", [MT, P, D], in_dt, kind="ExternalInput").ap()
    yt = nc.dram_tensor("yt", [DT, P, C], f32, kind="ExternalOutput").ap()

    sizes = _chunk_sizes(C)
    chunks = []
    off = 0
    for nn in sizes:
        chunks.append((off, nn))
        off += nn
    nn0 = chunks[0][1]
    silu = mybir.ActivationFunctionType.Silu

    with tile.TileContext(nc) as tc, ExitStack() as ctx:
        wpool = ctx.enter_context(tc.tile_pool(name="w", bufs=1))
        xpool = ctx.enter_context(tc.tile_pool(name="x", bufs=BUFS["x"]))
        gpool = ctx.enter_context(tc.tile_pool(name="g", bufs=BUFS["g"]))
        spool = ctx.enter_context(tc.tile_pool(name="s", bufs=BUFS["s"]))
        opool = ctx.enter_context(tc.tile_pool(name="o", bufs=BUFS["o"]))
        pspool = ctx.enter_context(
            tc.tile_pool(name="ps", bufs=BUFS["ps"], space="PSUM")
        )

        # weights at k-pair granularity (matches DMA granularity so tile
        # dependency tracking releases matmuls as each pair lands)
        w1p = [
            wpool.tile([P, 2, FP], in_dt, tag=f"w1_{kp}", name=f"w1_{kp}")
            for kp in range(KP)
        ]
        w2p = [
            wpool.tile([P, 2, FP], in_dt, tag=f"w2_{kp}", name=f"w2_{kp}")
            for kp in range(KP)
        ]
        w3_sb = wpool.tile([P, MT, D], in_dt, tag="w3", name="w3_sb")
        x0p = [
            xpool.tile([P, 2, nn0], in_dt, tag=f"x0_{kp}", name=f"x0_{kp}")
            for kp in range(KP)
        ]
        xn_t = [
            xpool.tile([P, KT, nn], in_dt, tag=f"xc_{c}", name=f"xc_{c}")
            for c, (_, nn) in enumerate(chunks)
            if c > 0
        ]

        def w1s(k, m):
            return w1p[k // 2][:, k % 2, m * P : (m + 1) * P]

        def w2s(k, m):
            return w2p[k // 2][:, k % 2, m * P : (m + 1) * P]

        def w3s(m, d):
            return w3_sb[:, m, d * P : (d + 1) * P]

        # dummy matmuls on a zeroed tile bridge the DMA-preload window so the
        # PE p-state ramp (full speed after 3us continuous busy) is already
        # paid when the first real matmul issues; outputs are never used
        warm = wpool.tile([P, WARM_COLS], in_dt, tag="warm", name="warm")
        nc.gpsimd.memset(warm[:], 0.0)
        wps = pspool.tile([P, WARM_COLS], f32, tag="ps", name="wps")
        for _ in range(WARMUP_MMS):
            nc.tensor.matmul(wps[:], warm[:, :P], warm[:], start=True, stop=True)
        wsink = wpool.tile([P, WARM_COLS], f32, tag="wsink", name="wsink")
        nc.scalar.copy(wsink[:], wps[:])  # consume so the PSUM slot frees

        # ---- preload: all loads on the SP queue, in consumption order ----
        for kp in range(KP):
            nc.sync.dma_start(
                w1p[kp][:], w1t[2 * kp : 2 * kp + 2].rearrange("k p f -> p k f")
            )
            nc.sync.dma_start(
                x0p[kp][:],
                xt[2 * kp : 2 * kp + 2, :, 0:nn0].rearrange("k p n -> p k n"),
            )
            nc.sync.dma_start(
                w2p[kp][:], w2t[2 * kp : 2 * kp + 2].rearrange("k p f -> p k f")
            )
        nc.sync.dma_start(w3_sb[:], w3t.rearrange("m p d -> p m d"))
        for c, (n0, nn) in enumerate(chunks):
            if c == 0:
                continue
            nc.sync.dma_start(
                xn_t[c - 1][:], xt[:, :, n0 : n0 + nn].rearrange("k p n -> p k n")
            )

        for c, (n0, nn) in enumerate(chunks):
            last = c == len(chunks) - 1

            def xs(k, c=c):
                if c == 0:
                    return x0p[k // 2][:, k % 2, :]
                return xn_t[c - 1][:, k, :]

            gs = []
            if c == 0:
                # fused W1+W2 k-loop over all 4 f-tiles: PE consumes one
                # (w1,x0,w2) k-pair DMA triplet per 16 matmuls, staying just
                # behind the HWDGE delivery rate during preload
                ps1 = {
                    m: pspool.tile([P, nn], f32, tag="ps", name=f"ps1_{m}")
                    for m in range(MT)
                }
                ps2 = {
                    m: pspool.tile([P, nn], f32, tag="ps", name=f"ps2_{m}")
                    for m in range(MT)
                }
                for k in range(KT):
                    for m in range(MT):
                        nc.tensor.matmul(
                            ps1[m][:], w1s(k, m), xs(k),
                            start=(k == 0), stop=(k == KT - 1),
                        )
                    for m in range(MT):
                        nc.tensor.matmul(
                            ps2[m][:], w2s(k, m), xs(k),
                            start=(k == 0), stop=(k == KT - 1),
                        )
                for m in range(MT):
                    sil = spool.tile([P, nn], f32, tag="sil", name="sil")
                    nc.scalar.activation(sil[:], ps1[m][:], silu)
                    g = gpool.tile([P, nn], in_dt, tag=f"g{m}", name=f"g{m}")
                    nc.vector.tensor_mul(g[:], sil[:], ps2[m][:])
                    gs.append(g)
            else:
                # steady state: m-pairs so only 4 PSUM banks are held at a
                # time, leaving headroom for stage-B / next-chunk overlap
                for mp in range(MT // 2):
                    ms = (2 * mp, 2 * mp + 1)
                    ps1 = {
                        m: pspool.tile([P, nn], f32, tag="ps", name=f"ps1_{m}")
                        for m in ms
                    }
                    for k in range(KT):
                        for m in ms:
                            nc.tensor.matmul(
                                ps1[m][:], w1s(k, m), xs(k),
                                start=(k == 0), stop=(k == KT - 1),
                            )
                    ps2 = {
                        m: pspool.tile([P, nn], f32, tag="ps", name=f"ps2_{m}")
                        for m in ms
                    }
                    for k in range(KT):
                        for m in ms:
                            nc.tensor.matmul(
                                ps2[m][:], w2s(k, m), xs(k),
                                start=(k == 0), stop=(k == KT - 1),
                            )
                    for m in ms:
                        sil = spool.tile([P, nn], f32, tag="sil", name="sil")
                        nc.scalar.activation(sil[:], ps1[m][:], silu)
                        g = gpool.tile([P, nn], in_dt, tag=f"g{m}", name=f"g{m}")
                        nc.vector.tensor_mul(g[:], sil[:], ps2[m][:])
                        gs.append(g)

            # ---- stage B: y^T[d] = sum_m W3[m,d]^T g[m] ----
            if not last:
                ot = opool.tile([P, DT, nn], f32, tag="ot", name="ot")
            store_engines = [nc.sync, nc.scalar, nc.gpsimd]
            for d in range(DT):
                pso = pspool.tile([P, nn], f32, tag="ps", name="pso")
                for m in range(MT):
                    nc.tensor.matmul(
                        pso[:], w3s(m, d), gs[m][:],
                        start=(m == 0), stop=(m == MT - 1),
                    )
                if last:
                    otd = opool.tile([P, nn], f32, tag=f"otd{d % 3}", name=f"otd{d}")
                    if d % 2 == 1:
                        nc.scalar.copy(otd[:], pso[:])
                    else:
                        nc.vector.tensor_copy(otd[:], pso[:])
                    # fan the tail stores across SP/Act/DVE issue queues so
                    # the final store chain isn't serialized on one SEQ
                    store_engines[d % 3].dma_start(yt[d, :, n0 : n0 + nn], otd[:])
                else:
                    if d % 2 == 1:
                        nc.scalar.copy(ot[:, d, :], pso[:])
                    else:
                        nc.vector.tensor_copy(ot[:, d, :], pso[:])
            if not last:
                # one merged store per chunk; gpsimd's SWDGE path skips the
                # shared HWDGE device and the Pool engine is otherwise idle
                eng = nc.gpsimd if c % 2 == 0 else nc.scalar
                eng.dma_start(
                    yt[:, :, n0 : n0 + nn].rearrange("d p n -> p d n"), ot[:]
                )

    nc.compile()
    return nc


LAST_RESULTS = None  # BassKernelResults of the most recent run (for test harness)


def kernel(x, Wg, bg, W1, W2, W3):
    global LAST_RESULTS
    from concourse.bass_utils import run_bass_kernel_spmd

    x = np.asarray(x)
    Wg, bg = np.asarray(Wg), np.asarray(bg)
    W1, W2, W3 = np.asarray(W1), np.asarray(W2), np.asarray(W3)
    B, S, d = x.shape
    T = B * S
    assert d == D and Wg.shape == (E, D)

    xf = np.ascontiguousarray(x.reshape(T, D))

    # ---- host gate + top-1 routing (fp64: exact vs any fp32 backend) ----
    gate = xf.astype(np.float64) @ Wg.astype(np.float64).T + bg.astype(np.float64)
    eid = np.argmax(gate, axis=1)
    counts = np.bincount(eid, minlength=E)
    order = np.argsort(eid, kind="stable")
    offs = np.concatenate(([0], np.cumsum(counts)))

    C = max(MIN_C, 2 * int(-(-counts.max() // 2)))
    key = (C, MM_MODE)
    if key not in _cache:
        _cache[key] = _build(C)
    nc = _cache[key]

    in_dt = _np_in_dtype()

    # ---- build per-core inputs (dispatch) ----
    in_maps = []
    tok_lists = []
    for e in range(E):
        toks = order[offs[e] : offs[e + 1]]
        tok_lists.append(toks)
        ce = len(toks)
        xeT = np.zeros((D, C), dtype=in_dt)
        if ce:
            xeT[:, :ce] = xf[toks].T.astype(in_dt)
        w1 = np.zeros((D, FP), dtype=in_dt)
        w1[:, :F] = W1[e].T.astype(in_dt)
        w2 = np.zeros((D, FP), dtype=in_dt)
        w2[:, :F] = W2[e].T.astype(in_dt)
        w3 = np.zeros((FP, D), dtype=in_dt)
        w3[:F, :] = W3[e].T.astype(in_dt)
        in_maps.append(
            {
                "xt": np.ascontiguousarray(xeT.reshape(KT, P, C)),
                "w1t": np.ascontiguousarray(w1.reshape(KT, P, FP)),
                "w2t": np.ascontiguousarray(w2.reshape(KT, P, FP)),
                "w3t": np.ascontiguousarray(w3.reshape(MT, P, D)),
            }
        )

    res = run_bass_kernel_spmd(nc, in_maps, list(range(E)))
    LAST_RESULTS = res

    # ---- combine: scatter outputs back to token order ----
    y = np.empty((T, D), dtype=np.float32)
    for e in range(E):
        toks = tok_lists[e]
        if len(toks):
            yte = res.results[e]["yt"].reshape(D, C)
            y[toks] = yte[:, : len(toks)].T
    return y.reshape(B, S, d)
